# revision 35
# baseline (speedup 1.0000x reference)
"""Chamfer distance kernel for 8 Trainium2 NeuronCores.

Problem: adv [4, 8192, 3], ori [4, 8192, 3], weights [4] ->
    scalar = mean_b( w_b * mean_n min_m d2(adv_bn, ori_bm)
                   + w_b * mean_m min_n d2(adv_bn, ori_bm) )

Sharding: data-parallel over B (4 batches) x 2-way split of N per batch
= 8 cores. Each core handles N_loc=4096 adv rows against all M=8192 ori
points of its batch.

Per-core dataflow (all on-chip; inputs are ~100KB):
  PE    : fp32 matmul with K=5 augmented operands computes
          s[n, m] = 2 x.y - |x|^2 - |y|^2 = -d2[n, m], 128x512 tiles
          into PSUM (4 banks per group, double-buffered).
  ACT   : copies PSUM fp32 -> SBUF fp16 (2048 wide).
  DVE   : per 128-row block: row-max of s (-> -min_m d2) via a 2x fp16
          max tree; running elementwise max into a [128, 8192]
          accumulator for the m-direction.
  Host  : builds augmented operands, gathers per-core results, finishes
          the 128-way cross-partition max (exact int16-view trick),
          negates, reduces to the scalar.
"""

import sys

if "/opt/trn_rl_repo" not in sys.path:
    sys.path.insert(0, "/opt/trn_rl_repo")

import numpy as np

B = 4
N = 8192
M = 8192
NCORES = 8
NLOC = N // 2          # rows per core
P = 128                # partitions
NB = NLOC // P         # 32 row blocks per core
K = 5                  # augmented contraction dim
MT = 512               # matmul moving free (1 PSUM bank of fp32)
MG = 2048              # ACT copy group (4 banks)
NG = M // MG           # 4 groups per row block

_NC = None

FP32R = False              # float32r matmuls need producer-side rounding;
                           # PE is not the bottleneck, so keep plain fp32


def _patch_tail_drain():
    """This walrus build rejects >2 sync waits on one CTRL instruction.
    Split the TileContext tail drain into one drain per pending proc."""
    import bass_rust
    from concourse import tile as _tile

    if getattr(_tile.TileContext, "_ant_drain_patched", False):
        return

    def _drain_and_barrier(self, tick_clock, wait_clock):
        gc = tick_clock.global_clock
        n = bass_rust.N_PROCS
        for p in range(n):
            if gc[p] == 0:
                continue
            sub = bass_rust.VectorClock(
                [gc[q] if q == p else 0 for q in range(n)]
            )
            d = self.nc.sync.drain()
            wait_clock.add_sem_waits(d.ins, _tile.ScopedClock({None: sub}))
        self.nc.all_engine_barrier()
        popped = self.nc._tile_sem_poison_stack.pop()
        assert popped is self._sem_poison
        self.nc.clear_and_free_semaphores(list(self.sems.allocated().values()))
        self.nc.all_engine_barrier()

    _tile.TileContext._drain_and_barrier = _drain_and_barrier
    _tile.TileContext._ant_drain_patched = True


def _strip_redundant_waits(nc):
    """Tile's emitted waits are not transitively minimal, and this walrus
    enforces tiny per-instruction sync-wait limits (Matmult LDW struct: 1,
    CTRL/ACTIVATE: 2). Drop waits provably implied through an
    engine-semaphore chain: a wait (S2, v2) on X is redundant when another
    wait (S1, v1) of X resolves at engine instruction A and A's engine
    prefix already waited for (S2, >= v2) or itself produced S2 >= v2.
    Engines complete in order (GPSIMD excluded - per-Q7 FIFO only)."""
    insts = []
    for f in nc.m.functions:
        for b in f.blocks:
            insts.extend(b.instructions)

    ordered_engines = ("PE", "Activation", "DVE", "SP")
    recs = []
    for i in insts:
        si = i.sync_info
        waits = [(w.ant_name, int(w.wait_value)) for w in si.on_wait] if si else []
        ups = [(u.ant_name, int(u.update_value)) for u in si.on_update] if si else []
        recs.append((i, str(i.engine).split(".")[-1], waits, ups))

    # per-sem cumulative increments in block order
    sem_hist = {}           # sem -> list of (value_after, inst_idx)
    cum = {}
    # per-instruction snapshots of its engine's prefix state
    seen_waits_at = []      # idx -> {sem: max waited value} for that engine
    produced_at = []        # idx -> {sem: cum produced by that engine}
    eng_seen = {}
    eng_prod = {}
    all_carriers = {}       # sem -> set of engine names carrying updates
    for idx, (i, eng, waits, ups) in enumerate(recs):
        sw = eng_seen.setdefault(eng, {})
        sp = eng_prod.setdefault(eng, {})
        for s, v in waits:
            if v > sw.get(s, -1):
                sw[s] = v
        for s, v in ups:
            inc = v if v > 0 else 1
            cum[s] = cum.get(s, 0) + inc
            all_carriers.setdefault(s, set()).add(eng)
            # only engine-own semaphores update at engine completion; DMA
            # sems ride the issuing instruction but fire asynchronously
            if s.rsplit("_", 1)[0] == eng:
                sem_hist.setdefault(s, []).append((cum[s], idx))
                sp[s] = cum[s]
        seen_waits_at.append(dict(sw))
        produced_at.append(dict(sp))

    def resolver(sem, val):
        """Index of the instruction whose completion first makes sem >= val,
        if sem is produced by a single in-order engine; else None."""
        hist = sem_hist.get(sem)
        if not hist:
            return None
        for v, idx in hist:
            if v >= val:
                eng = recs[idx][1]
                if eng not in ordered_engines:
                    return None
                # all producers of this sem must be on that engine
                if all_carriers.get(sem, set()) != {eng}:
                    return None
                return idx
        return None

    import bass_rust  # noqa: F401

    for idx, (i, eng, waits, ups) in enumerate(recs):
        if len(waits) < 2:
            continue
        si = i.sync_info
        keep = list(range(len(waits)))
        for a in range(len(waits)):
            if a not in keep:
                continue
            s1, v1 = waits[a]
            ai = resolver(s1, v1)
            if ai is None:
                continue
            for bq in list(keep):
                if bq == a:
                    continue
                s2, v2 = waits[bq]
                if (seen_waits_at[ai].get(s2, -1) >= v2
                        or produced_at[ai].get(s2, -1) >= v2):
                    keep.remove(bq)
        # strict-FIFO engines execute in order; a wait on the instruction's
        # own engine sem (produced solely by that engine) is redundant.
        # PE excluded: its queue reorders LDWEIGHTS ahead of matmuls.
        # Pool included: its ops here are identical-shape chains, ordered
        # per-Q7-FIFO partition-wise.
        if len(keep) >= 2 and eng in ("Activation", "DVE", "SP", "Pool"):
            for k in list(keep):
                s, v = waits[k]
                if len(keep) < 2:
                    break
                if (s.rsplit("_", 1)[0] == eng
                        and all_carriers.get(s, set()) == {eng}):
                    keep.remove(k)
        if len(keep) < len(waits):
            si.on_wait = [si.on_wait[k] for k in keep]
            i.sync_info = si


def _build_program():
    from concourse import bass, mybir, tile

    _patch_tail_drain()

    f32 = mybir.dt.float32
    f16 = mybir.dt.float16

    nc = bass.Bass("TRN2", target_bir_lowering=False, num_devices=NCORES)
    # lhsT and rhs packed in one tensor -> one input DMA -> one queue sem
    inp_d = nc.dram_tensor("inp", [K, NLOC + M], f32, kind="ExternalInput")
    xy_d = nc.dram_tensor("xy", [P, NB], f32, kind="ExternalOutput")
    yx_d = nc.dram_tensor("yx", [P, M], f16, kind="ExternalOutput")

    mx = mybir.AluOpType.max

    with tile.TileContext(nc) as tc:
        with (
            tc.tile_pool(name="const", bufs=1) as const,
            tc.tile_pool(name="psum", bufs=2, space="PSUM") as psum_pool,
            tc.tile_pool(name="buf", bufs=2) as bufp,
            tc.tile_pool(name="scr", bufs=2) as scrp,
        ):
            inp = const.tile([K, NLOC + M], f32)
            # single SWDGE-queue DMA: consumers wait on one DMA semaphore
            # (this walrus rejects >1 sync wait on a Matmult)
            nc.gpsimd.dma_start(out=inp, in_=inp_d[:, :])
            lhsT = inp[:, :NLOC]
            rhs = inp[:, NLOC:]
            yx_acc = const.tile([P, M], f16)
            xy_out = const.tile([P, NB], f32)
            warb = const.tile([1, 1], f16)
            nc.vector.memset(warb, 0.0)

            if FP32R:
                f32r = mybir.dt.float32r
                lhsT_mm = lhsT.bitcast(f32r)
                rhs_mm = rhs.bitcast(f32r)
            else:
                lhsT_mm = lhsT
                rhs_mm = rhs

            for i in range(NB):
                buf = bufp.tile([P, M], f16)
                s = scrp.tile([P, M // 2], f16)
                # WAR absorber chain: A takes the DVE wait (scratch slot's
                # previous readers cover buf's DVE readers too), B takes the
                # GPSIMD wait (its DVE wait elides via A). Keeps every real
                # copy within this walrus's 1-sync-wait ACTIVATE limit.
                nc.scalar.copy(s[0:1, 0:1], warb)
                nc.scalar.copy(buf[0:1, 0:1], s[0:1, 0:1])
                for g in range(NG):
                    pt = psum_pool.tile([P, MG], f32)
                    for j in range(NG):
                        m0 = g * MG + j * MT
                        nc.tensor.matmul(
                            pt[:, j * MT:(j + 1) * MT],
                            lhsT_mm[:, i * P:(i + 1) * P],
                            rhs_mm[:, m0:m0 + MT],
                            start=True,
                            stop=True,
                        )
                    nc.scalar.copy(buf[:, g * MG:(g + 1) * MG], pt)

                # m-direction (yx): running elementwise max over row blocks
                # (GPSIMD cannot run 2-input tensor ops on this toolchain,
                # so the whole chain lives on DVE at fp16 2x)
                if i == 0:
                    nc.vector.tensor_copy(yx_acc, buf)
                else:
                    nc.vector.tensor_max(yx_acc, yx_acc, buf)

                # n-direction (xy): row max via fp16 2x tree, then reduce
                nc.vector.tensor_max(s, buf[:, : M // 2], buf[:, M // 2:])
                w = M // 4
                while w >= MT:
                    nc.vector.tensor_max(s[:, :w], s[:, :w], s[:, w:2 * w])
                    w //= 2
                nc.vector.tensor_reduce(
                    out=xy_out[:, i:i + 1],
                    in_=s[:, :MT],
                    axis=mybir.AxisListType.X,
                    op=mx,
                )

            # the 128-way cross-partition max is finished on the host
            # (values are all <= 0, so an int16-view min is exact and fast)
            # tiny DVE write into yx_acc absorbs the GPSIMD wait, so the
            # final DMA resolves through the DVE semaphore alone
            nc.vector.tensor_scalar_max(
                yx_acc[0:1, M - 1:M], yx_acc[0:1, M - 1:M], -65504.0)
            nc.sync.dma_start(out=xy_d[:, :], in_=xy_out)
            nc.sync.dma_start(out=yx_d[:, :], in_=yx_acc)

    _strip_redundant_waits(nc)
    return nc


def _get_nc():
    global _NC
    if _NC is None:
        _NC = _build_program()
    return _NC


def _make_in_maps(adv, ori):
    adv = np.ascontiguousarray(adv, dtype=np.float32)
    ori = np.ascontiguousarray(ori, dtype=np.float32)
    in_maps = []
    for c in range(NCORES):
        b, half = divmod(c, 2)
        x = adv[b, half * NLOC:(half + 1) * NLOC]     # [NLOC, 3]
        y = ori[b]                                    # [M, 3]
        inp = np.empty((K, NLOC + M), np.float32)
        lhsT = inp[:, :NLOC]
        rhs = inp[:, NLOC:]
        lhsT[0:3] = x.T
        lhsT[3] = (x * x).sum(axis=1)
        lhsT[4] = 1.0
        rhs[0:3] = 2.0 * y.T
        rhs[3] = -1.0
        rhs[4] = -(y * y).sum(axis=1)
        in_maps.append({"inp": inp})
    return in_maps


def _combine(results, weights):
    loss = np.float32(0.0)
    for b in range(B):
        xy_parts = []
        yx_parts = []
        for half in range(2):
            r = results[2 * b + half]
            # xy[p, i] = max_m s for local row i*128+p  ->  -min_m d2
            xy_parts.append(-r["xy"].T.reshape(-1))
            # max over partitions of fp16 values <= 0 == int16-view min
            yx16 = r["yx"].view(np.int16).min(axis=0).view(np.float16)
            yx_parts.append(-yx16.astype(np.float32))
        dist_xy = np.concatenate(xy_parts)                  # [N]
        dist_yx = np.minimum(yx_parts[0], yx_parts[1])      # [M]
        w = np.float32(weights[b])
        loss += w * dist_xy.mean(dtype=np.float32)
        loss += w * dist_yx.mean(dtype=np.float32)
    return np.float32(loss / B)


def kernel(adv, ori, weights, _runner_kwargs=None):
    from concourse.bass_utils import run_bass_kernel_spmd

    nc = _get_nc()
    in_maps = _make_in_maps(adv, ori)
    kw = _runner_kwargs or {}
    res = run_bass_kernel_spmd(nc, in_maps, core_ids=list(range(NCORES)), **kw)
    out = _combine(res.results, np.asarray(weights))
    kernel.last_result = res
    return np.asarray(out, dtype=np.float32)
